# revision 1
# baseline (speedup 1.0000x reference)
"""DeepFM Trainium2 kernel (8-core data-parallel).

Math: x entries are binary {0,1}, so the per-feature embedding gather is
linear in x:  emb[b,f] = T0[f] + x[b,f]*(T1[f]-T0[f]).  The whole model
collapses to
    h1   = relu(x @ W1eff + b1eff)            # K=64 matmul (was K=1024)
    h2   = relu(h1 @ W2.T + b2)
    sum_e = x @ S + sbase                      # [B,16]
    cont  = x @ Wcont + bc                     # [B,32]
    col0  = x @ lin0 + c0 + 0.5*||sum_e||^2 - 0.5*||cont||^2
    out   = concat(col0, h2)
All weights precomputed on host (float64), device does two matmuls plus
elementwise drains.  Device output layout is transposed: [129, 8192] per
core (features on partitions); the host transposes back when unsharding.
"""

import numpy as np
import ml_dtypes

import concourse.bass as bass
import concourse.tile as tile
from concourse import bacc, mybir
from concourse.bass_utils import run_bass_kernel_spmd

B = 65536
FEAT = 64
NUM_DISC = 62
D = 16
H1, H2 = 256, 128
NCORES = 8
BS = B // NCORES          # 8192 rows per core
NSUP = BS // 1024         # 8 superblocks of 1024 rows (512 "A" + 512 "B")

F32 = mybir.dt.float32
BF16 = mybir.dt.bfloat16
AF = mybir.ActivationFunctionType
ALU = mybir.AluOpType

TRACE = False
TRACE_KW = {}
LAST_RESULT = None

_cached_nc = None


def _precompute_weights(emb_tables, Wc, bc, Wf, bf, W1, b1, W2, b2):
    """Host-side weight folding, float64 for exactness."""
    T = np.asarray(emb_tables, np.float64)        # [62, 2, 16]
    Wc = np.asarray(Wc, np.float64)               # [32, 2]
    bc = np.asarray(bc, np.float64)               # [32]
    Wf = np.asarray(Wf, np.float64)               # [1, 64]
    bf = np.asarray(bf, np.float64)               # [1]
    W1 = np.asarray(W1, np.float64)               # [256, 1024]
    b1 = np.asarray(b1, np.float64)               # [256]
    W2 = np.asarray(W2, np.float64)               # [128, 256]
    b2 = np.asarray(b2, np.float64)               # [128]

    A = np.zeros((64, 1024))
    base = np.zeros(1024)
    for f in range(NUM_DISC):
        A[f, 16 * f:16 * f + 16] = T[f, 1] - T[f, 0]
        base[16 * f:16 * f + 16] = T[f, 0]
    A[62, 992:1024] = Wc[:, 0]
    A[63, 992:1024] = Wc[:, 1]
    base[992:1024] = bc

    W1eff = A @ W1.T                              # [64, 256]
    b1eff = base @ W1.T + b1                      # [256]
    S = A.reshape(64, 64, 16).sum(axis=1)         # [64, 16]
    sbase = base.reshape(64, 16).sum(axis=0)      # [16]
    Wcont = A[:, 992:1024]                        # [64, 32]

    q0 = (T[:, 0] ** 2).sum(axis=1)               # [62]
    q1 = (T[:, 1] ** 2).sum(axis=1)
    qlin = np.zeros(64)
    qlin[:NUM_DISC] = q1 - q0
    qconst = q0.sum()
    lin0 = Wf[0] - 0.5 * qlin                     # [64]
    c0 = bf[0] - 0.5 * qconst                     # scalar

    def dup(a):  # stack A-copy (parts 0:64) and B-copy (parts 64:128)
        return np.concatenate([a, a], axis=0)

    wbig_h = np.zeros((64, 320))
    wbig_h[:, 0:128] = W1eff[:, 0:128]
    wbig_h[:, 128:256] = W1eff[:, 128:256]
    wbig_h[:, 256:272] = S
    wbig_h[:, 272:304] = Wcont
    wbig = dup(wbig_h).astype(ml_dtypes.bfloat16)          # [128, 320]

    w2t = np.zeros((128, 256))
    w2t[:, 0:128] = W2[:, 0:128].T
    w2t[:, 128:256] = W2[:, 128:256].T
    w2t = w2t.astype(ml_dtypes.bfloat16)                    # [128, 256]

    lin0_h = np.zeros((64, 32))
    lin0_h[:, 0] = lin0
    lin0w = dup(lin0_h).astype(ml_dtypes.bfloat16)          # [128, 32]
    coeff_h = np.zeros((64, 32))
    coeff_h[0:16, 0] = 0.5
    coeff_h[16:48, 0] = -0.5
    coeffw = dup(coeff_h).astype(ml_dtypes.bfloat16)        # [128, 32]

    b1w = np.stack([b1eff[0:128], b1eff[128:256]], axis=1).astype(np.float32)
    bext_h = np.zeros((64, 1))
    bext_h[0:16, 0] = sbase
    bext_h[16:48, 0] = bc
    bextw = dup(bext_h).astype(np.float32)                  # [128, 1]
    b2w = b2[:, None].astype(np.float32)                    # [128, 1]
    cbw = np.zeros((128, 1), np.float32)
    cbw[[0, 32, 64, 96], 0] = c0

    return dict(wbig=wbig, w2t=w2t, lin0w=lin0w, coeffw=coeffw,
                b1w=b1w, bextw=bextw, b2w=b2w, cbw=cbw)


def _build_nc():
    nc = bacc.Bacc(None, target_bir_lowering=False)

    x_d = nc.declare_dram_parameter("x", [BS, FEAT], F32, isOutput=False)
    wbig_d = nc.declare_dram_parameter("wbig", [128, 320], BF16, isOutput=False)
    w2t_d = nc.declare_dram_parameter("w2t", [128, 256], BF16, isOutput=False)
    lin0_d = nc.declare_dram_parameter("lin0w", [128, 32], BF16, isOutput=False)
    coeff_d = nc.declare_dram_parameter("coeffw", [128, 32], BF16, isOutput=False)
    b1_d = nc.declare_dram_parameter("b1w", [128, 2], F32, isOutput=False)
    bext_d = nc.declare_dram_parameter("bextw", [128, 1], F32, isOutput=False)
    b2_d = nc.declare_dram_parameter("b2w", [128, 1], F32, isOutput=False)
    cb_d = nc.declare_dram_parameter("cbw", [128, 1], F32, isOutput=False)
    outT_d = nc.declare_dram_parameter("outT", [129, BS], F32, isOutput=True)

    with tile.TileContext(nc) as tc:
        from contextlib import ExitStack
        with ExitStack() as ctx:
            constp = ctx.enter_context(tc.tile_pool(name="const", bufs=1))
            natp = ctx.enter_context(tc.tile_pool(name="nat", bufs=8))
            xtp = ctx.enter_context(tc.tile_pool(name="xt", bufs=8))
            h1p = ctx.enter_context(tc.tile_pool(name="h1", bufs=4))
            stkp = ctx.enter_context(tc.tile_pool(name="stk", bufs=2))
            colp = ctx.enter_context(tc.tile_pool(name="colsb", bufs=4))
            outp = ctx.enter_context(tc.tile_pool(name="outsb", bufs=8))
            pp0 = ctx.enter_context(
                tc.tile_pool(name="ps0", bufs=1, space=bass.MemorySpace.PSUM))
            pp1 = ctx.enter_context(
                tc.tile_pool(name="ps1", bufs=1, space=bass.MemorySpace.PSUM))
            ppe = ctx.enter_context(
                tc.tile_pool(name="pse", bufs=1, space=bass.MemorySpace.PSUM))
            pph = ctx.enter_context(
                tc.tile_pool(name="psh", bufs=1, space=bass.MemorySpace.PSUM))
            ppc = ctx.enter_context(
                tc.tile_pool(name="psc", bufs=1, space=bass.MemorySpace.PSUM))

            wbig = constp.tile([128, 320], BF16)
            nc.sync.dma_start(out=wbig[:], in_=wbig_d[:])
            w2t = constp.tile([128, 256], BF16)
            nc.sync.dma_start(out=w2t[:], in_=w2t_d[:])
            lin0 = constp.tile([128, 32], BF16)
            nc.sync.dma_start(out=lin0[:], in_=lin0_d[:])
            coeff = constp.tile([128, 32], BF16)
            nc.sync.dma_start(out=coeff[:], in_=coeff_d[:])
            b1 = constp.tile([128, 2], F32)
            nc.sync.dma_start(out=b1[:], in_=b1_d[:])
            bext = constp.tile([128, 1], F32)
            nc.sync.dma_start(out=bext[:], in_=bext_d[:])
            b2 = constp.tile([128, 1], F32)
            nc.sync.dma_start(out=b2[:], in_=b2_d[:])
            cb = constp.tile([128, 1], F32)
            nc.sync.dma_start(out=cb[:], in_=cb_d[:])

            pcol = None
            pcol_cols = []

            for g in range(NSUP):
                a0 = g * 1024  # batch offset; A = a0:a0+512, B = a0+512:a0+1024

                # --- load + cast x to bf16, 2 batch-halves interleaved ---
                # nat[p, 128*t + 64*half + c] = x[a0 + 512*half + 128*t + p, c]
                nat = natp.tile([128, 512], BF16)
                for t in range(4):
                    for half in range(2):
                        nc.gpsimd.dma_start(
                            out=nat[:, 128 * t + 64 * half:
                                    128 * t + 64 * half + 64],
                            in_=x_d[a0 + 512 * half + 128 * t:
                                    a0 + 512 * half + 128 * t + 128, :])

                # --- transpose: xt rows 0:64 = xT(A), rows 64:128 = xT(B) ---
                xt = xtp.tile([128, 512], BF16)
                for t in range(4):
                    nc.sync.dma_start(out=xt[:, 128 * t:128 * t + 128],
                                      in_=nat[:, 128 * t:128 * t + 128],
                                      transpose=True)

                # --- mm1: h1pre chunks + extras, A/B row-packed ---
                ps0 = pp0.tile([128, 1024], F32)
                ps1 = pp1.tile([128, 1024], F32)
                pse = ppe.tile([128, 512], F32)
                nc.tensor.matmul(ps0[:, 0:512], wbig[0:64, 0:128],
                                 xt[0:64, :], start=True, stop=True)
                nc.tensor.matmul(ps0[:, 512:1024], wbig[64:128, 0:128],
                                 xt[64:128, :], start=True, stop=True,
                                 tile_position=(64, 0))
                nc.tensor.matmul(ps1[:, 0:512], wbig[0:64, 128:256],
                                 xt[0:64, :], start=True, stop=True)
                nc.tensor.matmul(ps1[:, 512:1024], wbig[64:128, 128:256],
                                 xt[64:128, :], start=True, stop=True,
                                 tile_position=(64, 0))
                nc.tensor.matmul(pse[0:64, :], wbig[0:64, 256:320],
                                 xt[0:64, :], start=True, stop=True)
                nc.tensor.matmul(pse[64:128, :], wbig[64:128, 256:320],
                                 xt[64:128, :], start=True, stop=True,
                                 tile_position=(64, 64))

                # --- h1 relu drains (bias added here); bf16 out for mm2 ---
                h1c0 = h1p.tile([128, 1024], BF16, tag="h1")
                nc.scalar.activation(h1c0[:], ps0[:], AF.Relu,
                                     bias=b1[:, 0:1])
                h1c1 = h1p.tile([128, 1024], BF16, tag="h1")
                nc.vector.tensor_scalar(h1c1[:], ps1[:], b1[:, 1:2], 0.0,
                                        ALU.add, ALU.max)

                # --- extras: (z+bias)^2 for sum_e & cont rows ---
                stk = stkp.tile([128, 512], BF16)
                nc.scalar.activation(stk[:], pse[:], AF.Square,
                                     bias=bext[:])

                # --- FM scalar column: quadrant-packed K=64 matmuls ---
                if g % 2 == 0:
                    pcol = ppc.tile([128, 512], F32)
                    pcol_cols = []
                q = (g % 2) * 64
                pcol_cols += [(q, a0), (q + 32, a0 + 512)]
                nc.tensor.matmul(pcol[q:q + 32, :], coeff[0:64, :],
                                 stk[0:64, :], start=True, stop=False,
                                 tile_position=(0, q), skip_group_check=True)
                nc.tensor.matmul(pcol[q:q + 32, :], lin0[0:64, :],
                                 xt[0:64, :], start=False, stop=True,
                                 tile_position=(0, q), skip_group_check=True)
                nc.tensor.matmul(pcol[q + 32:q + 64, :], coeff[64:128, :],
                                 stk[64:128, :], start=True, stop=False,
                                 tile_position=(64, q + 32),
                                 skip_group_check=True)
                nc.tensor.matmul(pcol[q + 32:q + 64, :], lin0[64:128, :],
                                 xt[64:128, :], start=False, stop=True,
                                 tile_position=(64, q + 32),
                                 skip_group_check=True)

                # --- mm2: h2 = relu(h1 @ W2T + b2), A and B halves ---
                ph = pph.tile([128, 1024], F32)
                nc.tensor.matmul(ph[:, 0:512], w2t[:, 0:128],
                                 h1c0[:, 0:512], start=True, stop=False)
                nc.tensor.matmul(ph[:, 0:512], w2t[:, 128:256],
                                 h1c1[:, 0:512], start=False, stop=True)
                nc.tensor.matmul(ph[:, 512:1024], w2t[:, 0:128],
                                 h1c0[:, 512:1024], start=True, stop=False)
                nc.tensor.matmul(ph[:, 512:1024], w2t[:, 128:256],
                                 h1c1[:, 512:1024], start=False, stop=True)

                outsb = outp.tile([128, 1024], F32)
                nc.scalar.activation(outsb[:, 0:512], ph[:, 0:512], AF.Relu,
                                     bias=b2[:])
                nc.vector.tensor_scalar(outsb[:, 512:1024], ph[:, 512:1024],
                                        b2[:], 0.0, ALU.add, ALU.max)
                nc.sync.dma_start(out=outT_d[1:129, a0:a0 + 1024],
                                  in_=outsb[:])

                # --- drain FM column every 2 superblocks ---
                if g % 2 == 1:
                    colsb = colp.tile([128, 512], F32)
                    nc.vector.tensor_scalar(colsb[:], pcol[:], cb[:], None,
                                            ALU.add)
                    for (row, c0_) in pcol_cols:
                        nc.sync.dma_start(
                            out=outT_d[0:1, c0_:c0_ + 512],
                            in_=colsb[row:row + 1, :])

    nc.compile()
    return nc


def kernel(x, emb_tables, Wc, bc, Wf, bf, W1, b1, W2, b2):
    global _cached_nc, LAST_RESULT
    w = _precompute_weights(emb_tables, Wc, bc, Wf, bf, W1, b1, W2, b2)
    if _cached_nc is None:
        _cached_nc = _build_nc()
    nc = _cached_nc

    x = np.ascontiguousarray(np.asarray(x, np.float32))
    in_maps = []
    for i in range(NCORES):
        m = {"x": x[i * BS:(i + 1) * BS]}
        m.update(w)
        in_maps.append(m)

    res = run_bass_kernel_spmd(nc, in_maps, list(range(NCORES)),
                               trace=TRACE, **TRACE_KW)
    LAST_RESULT = res
    out = np.empty((B, 129), np.float32)
    for i in range(NCORES):
        out[i * BS:(i + 1) * BS, :] = res.results[i]["outT"].T
    return out



# revision 2
# speedup vs baseline: 2.7943x; 2.7943x over previous
"""DeepFM Trainium2 kernel (8-core data-parallel), v2.

Math: x entries are binary {0,1}, so the per-feature embedding gather is
linear in x:  emb[b,f] = T0[f] + x[b,f]*(T1[f]-T0[f]).  The model folds to
    h1    = relu(x @ W1eff + b1eff)           # K=64 matmul
    h2    = relu(h1 @ W2.T + b2)
    fm    = x@lin0 + c0 + 0.5*||x@S+sbase||^2 - 0.5*||x@Wcont+bc||^2

Device structure per 1024-row superblock (A=rows 0:512, B=rows 512:1024;
x^T is prepared host-side as bf16 [128, 4096] with A-features on
partitions 0:64 and B-features on 64:128):
  - mm1: 4 matmuls K=64 (A/B pairs run concurrently via tile_position)
    -> two PSUM regions [128, 1024] (h1 dims 0:128, 128:256)
  - extras: ONE K=128 block-diag matmul -> pse[0:100] = per-half
    [sum_e(16) | cont(32) | lin+-(2)]; the linear fm term rides along as
    0.5*((z+.5)^2 - (z-.5)^2) = z through the square drain
  - drains: ACT relu-drains h1 chunk1 + squares extras; DVE relu-drains
    h1 chunk2 + the h2 output
  - fm: ONE K=100 matmul with +-0.5 coefficients accumulating into a
    persistent PSUM bank (cols 2g, 2g+1 select the superblock's rows)
  - mm2: 4 matmuls K=128 -> h2
Outputs: hT bf16 [128, 8192] (h2 transposed), fmv f32 [16, 512].
Host adds c0 to fm and transposes hT back. PSUM: 2+2+1+2+1 = 8 banks.
"""

import numpy as np
import ml_dtypes

import concourse.bass as bass
import concourse.tile as tile
from concourse import bacc, mybir
from concourse.bass_utils import run_bass_kernel_spmd

B = 65536
FEAT = 64
NUM_DISC = 62
D = 16
H1, H2 = 256, 128
NCORES = 8
BS = B // NCORES          # 8192 rows per core
NSUP = BS // 1024         # 8 superblocks of 1024 rows (512 "A" + 512 "B")

F32 = mybir.dt.float32
BF16 = mybir.dt.bfloat16
AF = mybir.ActivationFunctionType
ALU = mybir.AluOpType

TRACE = False
TRACE_KW = {}
LAST_RESULT = None

_cached_nc = None


def _precompute_weights(emb_tables, Wc, bc, Wf, bf, W1, b1, W2, b2):
    """Host-side weight folding, float64 for exactness."""
    T = np.asarray(emb_tables, np.float64)        # [62, 2, 16]
    Wc = np.asarray(Wc, np.float64)               # [32, 2]
    bc = np.asarray(bc, np.float64)               # [32]
    Wf = np.asarray(Wf, np.float64)               # [1, 64]
    bf = np.asarray(bf, np.float64)               # [1]
    W1 = np.asarray(W1, np.float64)               # [256, 1024]
    b1 = np.asarray(b1, np.float64)               # [256]
    W2 = np.asarray(W2, np.float64)               # [128, 256]
    b2 = np.asarray(b2, np.float64)               # [128]

    A = np.zeros((64, 1024))
    base = np.zeros(1024)
    for f in range(NUM_DISC):
        A[f, 16 * f:16 * f + 16] = T[f, 1] - T[f, 0]
        base[16 * f:16 * f + 16] = T[f, 0]
    A[62, 992:1024] = Wc[:, 0]
    A[63, 992:1024] = Wc[:, 1]
    base[992:1024] = bc

    W1eff = A @ W1.T                              # [64, 256]
    b1eff = base @ W1.T + b1                      # [256]
    S = A.reshape(64, 64, 16).sum(axis=1)         # [64, 16]
    sbase = base.reshape(64, 16).sum(axis=0)      # [16]
    Wcont = A[:, 992:1024]                        # [64, 32]

    q0 = (T[:, 0] ** 2).sum(axis=1)               # [62]
    q1 = (T[:, 1] ** 2).sum(axis=1)
    qlin = np.zeros(64)
    qlin[:NUM_DISC] = q1 - q0
    qconst = q0.sum()
    lin0 = Wf[0] - 0.5 * qlin                     # [64]
    c0 = bf[0] - 0.5 * qconst                     # scalar

    def dup(a):  # stack A-copy (parts 0:64) and B-copy (parts 64:128)
        return np.concatenate([a, a], axis=0)

    wmm1 = dup(W1eff).astype(ml_dtypes.bfloat16)           # [128, 256]

    # extras block: per half [S(16) | Wcont(32) | lin0 | lin0] = 50 cols
    blkA = np.concatenate(
        [S, Wcont, lin0[:, None], lin0[:, None]], axis=1)  # [64, 50]
    wext = np.zeros((128, 100))
    wext[0:64, 0:50] = blkA
    wext[64:128, 50:100] = blkA
    wext = wext.astype(ml_dtypes.bfloat16)

    # extras bias: sum_e -> sbase, cont -> bc, lin rows -> +-0.5
    bx = np.concatenate([sbase, bc, [0.5], [-0.5]])        # [50]
    bext = np.zeros((128, 1), np.float32)
    bext[0:50, 0] = bx
    bext[50:100, 0] = bx

    # fm coefficients: 0.5*sum_e^2 - 0.5*cont^2 + 0.5*((z+.5)^2-(z-.5)^2)
    cf = np.zeros(50)
    cf[0:16] = 0.5
    cf[16:48] = -0.5
    cf[48] = 0.5
    cf[49] = -0.5
    wcoef = np.zeros((128, 16 * NSUP))
    for g in range(NSUP):
        wcoef[0:50, 16 * g + 2 * g] = cf
        wcoef[50:100, 16 * g + 2 * g + 1] = cf
    wcoef = wcoef.astype(ml_dtypes.bfloat16)               # [128, 128]

    w2t = np.zeros((128, 256))
    w2t[:, 0:128] = W2[:, 0:128].T
    w2t[:, 128:256] = W2[:, 128:256].T
    w2t = w2t.astype(ml_dtypes.bfloat16)                   # [128, 256]

    b1w = np.stack([b1eff[0:128], b1eff[128:256]], axis=1).astype(np.float32)
    b2w = b2[:, None].astype(np.float32)                   # [128, 1]

    w = dict(wmm1=wmm1, wext=wext, bextw=bext, wcoef=wcoef,
             w2t=w2t, b1w=b1w, b2w=b2w)
    return w, float(c0)


def _pack_x(x):
    """x [B, 64] f32 -> per-core xtd [128, 4096] bf16 with
    xtd[c][64*h + f, 512*g + j] = x[c*8192 + 1024*g + 512*h + j, f]."""
    xc = np.asarray(x, np.float32).reshape(NCORES, NSUP, 2, 512, FEAT)
    xt = xc.transpose(0, 2, 4, 1, 3).reshape(NCORES, 128, NSUP * 512)
    return np.ascontiguousarray(xt).astype(ml_dtypes.bfloat16)


def _build_nc():
    nc = bacc.Bacc(None, target_bir_lowering=False)

    xtd_d = nc.declare_dram_parameter("xtd", [128, 512 * NSUP], BF16,
                                      isOutput=False)
    wmm1_d = nc.declare_dram_parameter("wmm1", [128, 256], BF16, isOutput=False)
    wext_d = nc.declare_dram_parameter("wext", [128, 100], BF16, isOutput=False)
    wcoef_d = nc.declare_dram_parameter("wcoef", [128, 16 * NSUP], BF16,
                                        isOutput=False)
    w2t_d = nc.declare_dram_parameter("w2t", [128, 256], BF16, isOutput=False)
    b1_d = nc.declare_dram_parameter("b1w", [128, 2], F32, isOutput=False)
    bext_d = nc.declare_dram_parameter("bextw", [128, 1], F32, isOutput=False)
    b2_d = nc.declare_dram_parameter("b2w", [128, 1], F32, isOutput=False)
    hT_d = nc.declare_dram_parameter("hT", [128, BS], BF16, isOutput=True)
    fmv_d = nc.declare_dram_parameter("fmv", [16, 512], F32, isOutput=True)

    with tile.TileContext(nc) as tc:
        from contextlib import ExitStack
        with ExitStack() as ctx:
            constp = ctx.enter_context(tc.tile_pool(name="const", bufs=1))
            xtp = ctx.enter_context(tc.tile_pool(name="xt", bufs=NSUP))
            h1p = ctx.enter_context(tc.tile_pool(name="h1", bufs=4))
            stkp = ctx.enter_context(tc.tile_pool(name="stk", bufs=2))
            outp = ctx.enter_context(tc.tile_pool(name="outsb", bufs=4))
            colp = ctx.enter_context(tc.tile_pool(name="colsb", bufs=1))
            pp1 = ctx.enter_context(
                tc.tile_pool(name="ps1", bufs=1, space=bass.MemorySpace.PSUM))
            pp2 = ctx.enter_context(
                tc.tile_pool(name="ps2", bufs=1, space=bass.MemorySpace.PSUM))
            ppe = ctx.enter_context(
                tc.tile_pool(name="pse", bufs=1, space=bass.MemorySpace.PSUM))
            pph = ctx.enter_context(
                tc.tile_pool(name="psh", bufs=1, space=bass.MemorySpace.PSUM))
            ppc = ctx.enter_context(
                tc.tile_pool(name="psc", bufs=1, space=bass.MemorySpace.PSUM))

            wmm1 = constp.tile([128, 256], BF16)
            nc.sync.dma_start(out=wmm1[:], in_=wmm1_d[:])
            wext = constp.tile([128, 100], BF16)
            nc.sync.dma_start(out=wext[:], in_=wext_d[:])
            wcoef = constp.tile([128, 16 * NSUP], BF16)
            nc.sync.dma_start(out=wcoef[:], in_=wcoef_d[:])
            w2t = constp.tile([128, 256], BF16)
            nc.sync.dma_start(out=w2t[:], in_=w2t_d[:])
            b1 = constp.tile([128, 2], F32)
            nc.sync.dma_start(out=b1[:], in_=b1_d[:])
            bext = constp.tile([128, 1], F32)
            nc.sync.dma_start(out=bext[:], in_=bext_d[:])
            b2 = constp.tile([128, 1], F32)
            nc.sync.dma_start(out=b2[:], in_=b2_d[:])

            # fm accumulator, persistent across all superblocks
            pcol = ppc.tile([16, 512], F32)

            for g in range(NSUP):
                xt = xtp.tile([128, 512], BF16)
                nc.sync.dma_start(out=xt[:],
                                  in_=xtd_d[:, 512 * g:512 * (g + 1)])

                # --- mm1: h1pre, A/B halves concurrent per chunk ---
                ps1t = pp1.tile([128, 1024], F32)
                nc.tensor.matmul(ps1t[:, 0:512], wmm1[0:64, 0:128],
                                 xt[0:64, :], start=True, stop=True)
                nc.tensor.matmul(ps1t[:, 512:1024], wmm1[64:128, 0:128],
                                 xt[64:128, :], start=True, stop=True,
                                 tile_position=(64, 0))
                ps2t = pp2.tile([128, 1024], F32)
                nc.tensor.matmul(ps2t[:, 0:512], wmm1[0:64, 128:256],
                                 xt[0:64, :], start=True, stop=True)
                nc.tensor.matmul(ps2t[:, 512:1024], wmm1[64:128, 128:256],
                                 xt[64:128, :], start=True, stop=True,
                                 tile_position=(64, 0))

                # --- extras: one K=128 block-diag matmul ---
                pse = ppe.tile([128, 512], F32)
                nc.tensor.matmul(pse[0:100, :], wext[:, :], xt[:, :],
                                 start=True, stop=True)

                # --- h1 relu drains (bias fused); bf16 out for mm2 ---
                h1c0 = h1p.tile([128, 1024], BF16, tag="h1")
                nc.scalar.activation(h1c0[:], ps1t[:], AF.Relu,
                                     bias=b1[:, 0:1])
                h1c1 = h1p.tile([128, 1024], BF16, tag="h1")
                nc.vector.tensor_scalar(h1c1[:], ps2t[:], b1[:, 1:2], 0.0,
                                        ALU.add, ALU.max)

                # --- extras: (z+bias)^2 ---
                stk = stkp.tile([128, 512], BF16)
                nc.scalar.activation(stk[0:100, :], pse[0:100, :], AF.Square,
                                     bias=bext[0:100, 0:1])

                # --- fm: accumulate +-0.5 coefficient reduction ---
                nc.tensor.matmul(pcol[:], wcoef[0:100, 16 * g:16 * g + 16],
                                 stk[0:100, :],
                                 start=(g == 0), stop=(g == NSUP - 1),
                                 skip_group_check=True)

                # --- mm2: h2 = relu(h1 @ W2T + b2) ---
                ph = pph.tile([128, 1024], F32)
                nc.tensor.matmul(ph[:, 0:512], w2t[:, 0:128],
                                 h1c0[:, 0:512], start=True, stop=False)
                nc.tensor.matmul(ph[:, 0:512], w2t[:, 128:256],
                                 h1c1[:, 0:512], start=False, stop=True)
                nc.tensor.matmul(ph[:, 512:1024], w2t[:, 0:128],
                                 h1c0[:, 512:1024], start=True, stop=False)
                nc.tensor.matmul(ph[:, 512:1024], w2t[:, 128:256],
                                 h1c1[:, 512:1024], start=False, stop=True)

                outsb = outp.tile([128, 1024], BF16)
                nc.vector.tensor_scalar(outsb[:], ph[:], b2[:, 0:1], 0.0,
                                        ALU.add, ALU.max)
                nc.scalar.dma_start(out=hT_d[:, 1024 * g:1024 * (g + 1)],
                                    in_=outsb[:])

            # --- fm column drain, once ---
            colsb = colp.tile([16, 512], F32)
            nc.scalar.copy(colsb[:], pcol[:])
            nc.scalar.dma_start(out=fmv_d[:], in_=colsb[:])

    nc.compile()
    return nc


def kernel(x, emb_tables, Wc, bc, Wf, bf, W1, b1, W2, b2):
    global _cached_nc, LAST_RESULT
    w, c0 = _precompute_weights(emb_tables, Wc, bc, Wf, bf, W1, b1, W2, b2)
    if _cached_nc is None:
        _cached_nc = _build_nc()
    nc = _cached_nc

    xtd = _pack_x(x)
    in_maps = []
    for i in range(NCORES):
        m = {"xtd": xtd[i]}
        m.update(w)
        in_maps.append(m)

    res = run_bass_kernel_spmd(nc, in_maps, list(range(NCORES)),
                               trace=TRACE, **TRACE_KW)
    LAST_RESULT = res
    out = np.empty((B, 129), np.float32)
    for i in range(NCORES):
        r = res.results[i]
        out[i * BS:(i + 1) * BS, 0] = (
            r["fmv"].astype(np.float32).reshape(-1) + c0)
        out[i * BS:(i + 1) * BS, 1:129] = r["hT"].astype(np.float32).T
    return out


# revision 10
# speedup vs baseline: 3.5946x; 1.2864x over previous
"""DeepFM Trainium2 kernel (8-core data-parallel), v2.

Math: x entries are binary {0,1}, so the per-feature embedding gather is
linear in x:  emb[b,f] = T0[f] + x[b,f]*(T1[f]-T0[f]).  The model folds to
    h1    = relu(x @ W1eff + b1eff)           # K=64 matmul
    h2    = relu(h1 @ W2.T + b2)
    fm    = x@lin0 + c0 + 0.5*||x@S+sbase||^2 - 0.5*||x@Wcont+bc||^2

Device structure per 1024-row superblock (A=rows 0:512, B=rows 512:1024;
x^T is prepared host-side as bf16 [128, 4096] with A-features on
partitions 0:64 and B-features on 64:128):
  - mm1: 4 matmuls K=64 (A/B pairs run concurrently via tile_position)
    -> two PSUM regions [128, 1024] (h1 dims 0:128, 128:256)
  - extras: ONE K=128 block-diag matmul -> pse[0:100] = per-half
    [sum_e(16) | cont(32) | lin+-(2)]; the linear fm term rides along as
    0.5*((z+.5)^2 - (z-.5)^2) = z through the square drain
  - drains: ACT relu-drains h1 chunk1 + squares extras; DVE relu-drains
    h1 chunk2 + the h2 output
  - fm: ONE K=100 matmul with +-0.5 coefficients accumulating into a
    persistent PSUM bank (cols 2g, 2g+1 select the superblock's rows)
  - mm2: 4 matmuls K=128 -> h2
Outputs: hT bf16 [128, 8192] (h2 transposed), fmv f32 [16, 512].
Host adds c0 to fm and transposes hT back. PSUM: 2+2+1+2+1 = 8 banks.
"""

import numpy as np
import ml_dtypes

import concourse.bass as bass
import concourse.tile as tile
from concourse import bacc, mybir
from concourse.bass_utils import run_bass_kernel_spmd

B = 65536
FEAT = 64
NUM_DISC = 62
D = 16
H1, H2 = 256, 128
NCORES = 8
BS = B // NCORES          # 8192 rows per core
NSUP = BS // 1024         # 8 superblocks of 1024 rows (512 "A" + 512 "B")

F32 = mybir.dt.float32
BF16 = mybir.dt.bfloat16
FP8 = mybir.dt.float8e4
AF = mybir.ActivationFunctionType
ALU = mybir.AluOpType
SW2 = 64.0                # fp8 scale on W2 (host divides hT by SW2)
OSPLIT = 320              # out-drain columns drained on ACT (rest on DVE)

TRACE = False
TRACE_KW = {}
LAST_RESULT = None

_cached_nc = None


def _precompute_weights(emb_tables, Wc, bc, Wf, bf, W1, b1, W2, b2):
    """Host-side weight folding, float64 for exactness."""
    T = np.asarray(emb_tables, np.float64)        # [62, 2, 16]
    Wc = np.asarray(Wc, np.float64)               # [32, 2]
    bc = np.asarray(bc, np.float64)               # [32]
    Wf = np.asarray(Wf, np.float64)               # [1, 64]
    bf = np.asarray(bf, np.float64)               # [1]
    W1 = np.asarray(W1, np.float64)               # [256, 1024]
    b1 = np.asarray(b1, np.float64)               # [256]
    W2 = np.asarray(W2, np.float64)               # [128, 256]
    b2 = np.asarray(b2, np.float64)               # [128]

    A = np.zeros((64, 1024))
    base = np.zeros(1024)
    for f in range(NUM_DISC):
        A[f, 16 * f:16 * f + 16] = T[f, 1] - T[f, 0]
        base[16 * f:16 * f + 16] = T[f, 0]
    A[62, 992:1024] = Wc[:, 0]
    A[63, 992:1024] = Wc[:, 1]
    base[992:1024] = bc

    W1eff = A @ W1.T                              # [64, 256]
    b1eff = base @ W1.T + b1                      # [256]
    S = A.reshape(64, 64, 16).sum(axis=1)         # [64, 16]
    sbase = base.reshape(64, 16).sum(axis=0)      # [16]
    Wcont = A[:, 992:1024]                        # [64, 32]

    q0 = (T[:, 0] ** 2).sum(axis=1)               # [62]
    q1 = (T[:, 1] ** 2).sum(axis=1)
    qlin = np.zeros(64)
    qlin[:NUM_DISC] = q1 - q0
    qconst = q0.sum()
    lin0 = Wf[0] - 0.5 * qlin                     # [64]
    c0 = bf[0] - 0.5 * qconst                     # scalar

    def dup(a):  # stack A-copy (parts 0:64) and B-copy (parts 64:128)
        return np.concatenate([a, a], axis=0)

    wmm1 = dup(W1eff).astype(ml_dtypes.bfloat16)           # [128, 256]

    # extras block: per half [S(16) | Wcont(32) | lin0 | lin0] = 50 cols
    blkA = np.concatenate(
        [S, Wcont, lin0[:, None], lin0[:, None]], axis=1)  # [64, 50]
    wext = np.zeros((128, 100))
    wext[0:64, 0:50] = blkA
    wext[64:128, 50:100] = blkA
    wext = wext.astype(ml_dtypes.bfloat16)

    # extras bias: sum_e -> sbase, cont -> bc, lin rows -> +-0.5
    bx = np.concatenate([sbase, bc, [0.5], [-0.5]])        # [50]
    bext = np.zeros((128, 1), np.float32)
    bext[0:50, 0] = bx
    bext[50:100, 0] = bx

    # fm coefficients: 0.5*sum_e^2 - 0.5*cont^2 + 0.5*((z+.5)^2-(z-.5)^2)
    cf = np.zeros(50)
    cf[0:16] = 0.5
    cf[16:48] = -0.5
    cf[48] = 0.5
    cf[49] = -0.5
    wcoef = np.zeros((128, 16 * NSUP))
    for g in range(NSUP):
        wcoef[0:50, 16 * g + 2 * g] = cf
        wcoef[50:100, 16 * g + 2 * g + 1] = cf
    wcoef = wcoef.astype(ml_dtypes.bfloat16)               # [128, 128]

    # mm2 stationary for fp8 DoubleRow: w2q[p, s, m] = SW2 * W2[m, 128s+p]
    w2q = SW2 * np.stack([W2[:, 0:128].T, W2[:, 128:256].T], axis=1)
    w2q = w2q.astype(ml_dtypes.float8_e4m3)                # [128, 2, 128]

    b1w = np.stack([b1eff[0:128], b1eff[128:256]], axis=1).astype(np.float32)
    b2w = (SW2 * b2)[:, None].astype(np.float32)           # [128, 1]

    w = dict(wmm1=wmm1, wext=wext, bextw=bext, wcoef=wcoef,
             w2q=w2q, b1w=b1w, b2w=b2w)
    return w, float(c0)


def _pack_x(x):
    """x [B, 64] f32 -> per-core xtd [128, 4096] bf16 with
    xtd[c][64*h + f, 512*g + j] = x[c*8192 + 1024*g + 512*h + j, f]."""
    xc = np.asarray(x, np.float32).reshape(NCORES, NSUP, 2, 512, FEAT)
    xt = xc.transpose(0, 2, 4, 1, 3).reshape(NCORES, 128, NSUP * 512)
    return np.ascontiguousarray(xt).astype(ml_dtypes.bfloat16)


def _build_nc():
    nc = bacc.Bacc(None, target_bir_lowering=False)

    xtd_d = nc.declare_dram_parameter("xtd", [128, 512 * NSUP], BF16,
                                      isOutput=False)
    wmm1_d = nc.declare_dram_parameter("wmm1", [128, 256], BF16, isOutput=False)
    wext_d = nc.declare_dram_parameter("wext", [128, 100], BF16, isOutput=False)
    wcoef_d = nc.declare_dram_parameter("wcoef", [128, 16 * NSUP], BF16,
                                        isOutput=False)
    w2q_d = nc.declare_dram_parameter("w2q", [128, 2, 128], FP8, isOutput=False)
    b1_d = nc.declare_dram_parameter("b1w", [128, 2], F32, isOutput=False)
    bext_d = nc.declare_dram_parameter("bextw", [128, 1], F32, isOutput=False)
    b2_d = nc.declare_dram_parameter("b2w", [128, 1], F32, isOutput=False)
    hT_d = nc.declare_dram_parameter("hT", [128, BS], BF16, isOutput=True)
    fmv_d = nc.declare_dram_parameter("fmv", [16, 512], F32, isOutput=True)

    with tile.TileContext(nc) as tc:
        from contextlib import ExitStack
        with ExitStack() as ctx:
            constp = ctx.enter_context(tc.tile_pool(name="const", bufs=1))
            xtp = ctx.enter_context(tc.tile_pool(name="xt", bufs=NSUP))
            h1p = ctx.enter_context(tc.tile_pool(name="h1", bufs=4))
            stkp = ctx.enter_context(tc.tile_pool(name="stk", bufs=2))
            outp = ctx.enter_context(tc.tile_pool(name="outsb", bufs=4))
            colp = ctx.enter_context(tc.tile_pool(name="colsb", bufs=1))
            pp1 = ctx.enter_context(
                tc.tile_pool(name="ps1", bufs=1, space=bass.MemorySpace.PSUM))
            pp2 = ctx.enter_context(
                tc.tile_pool(name="ps2", bufs=1, space=bass.MemorySpace.PSUM))
            ppe = ctx.enter_context(
                tc.tile_pool(name="pse", bufs=1, space=bass.MemorySpace.PSUM))
            pph = ctx.enter_context(
                tc.tile_pool(name="psh", bufs=1, space=bass.MemorySpace.PSUM))
            ppc = ctx.enter_context(
                tc.tile_pool(name="psc", bufs=1, space=bass.MemorySpace.PSUM))

            wmm1 = constp.tile([128, 256], BF16)
            nc.sync.dma_start(out=wmm1[:], in_=wmm1_d[:])
            wext = constp.tile([128, 100], BF16)
            nc.sync.dma_start(out=wext[:], in_=wext_d[:])
            wcoef = constp.tile([128, 16 * NSUP], BF16)
            nc.sync.dma_start(out=wcoef[:], in_=wcoef_d[:])
            w2q = constp.tile([128, 2, 128], FP8)
            nc.sync.dma_start(out=w2q[:], in_=w2q_d[:])
            b1 = constp.tile([128, 2], F32)
            nc.sync.dma_start(out=b1[:], in_=b1_d[:])
            bext = constp.tile([128, 1], F32)
            nc.sync.dma_start(out=bext[:], in_=bext_d[:])
            b2 = constp.tile([128, 1], F32)
            nc.sync.dma_start(out=b2[:], in_=b2_d[:])

            # fm accumulator, persistent across all superblocks
            pcol = ppc.tile([16, 512], F32)

            # hoist all input loads so no store can queue ahead of them
            xts = []
            for g in range(NSUP):
                xt = xtp.tile([128, 512], BF16)
                nc.sync.dma_start(out=xt[:],
                                  in_=xtd_d[:, 512 * g:512 * (g + 1)])
                xts.append(xt)

            for g in range(NSUP):
                xt = xts[g]

                # --- mm1: h1pre, A/B halves concurrent per chunk ---
                ps1t = pp1.tile([128, 2, 512], F32)
                nc.tensor.matmul(ps1t[:, 0, :], wmm1[0:64, 0:128],
                                 xt[0:64, :], start=True, stop=True)
                nc.tensor.matmul(ps1t[:, 1, :], wmm1[64:128, 0:128],
                                 xt[64:128, :], start=True, stop=True,
                                 tile_position=(64, 0))
                ps2t = pp2.tile([128, 2, 512], F32)
                nc.tensor.matmul(ps2t[:, 0, :], wmm1[0:64, 128:256],
                                 xt[0:64, :], start=True, stop=True)
                nc.tensor.matmul(ps2t[:, 1, :], wmm1[64:128, 128:256],
                                 xt[64:128, :], start=True, stop=True,
                                 tile_position=(64, 0))

                # --- extras: one K=128 block-diag matmul ---
                pse = ppe.tile([128, 512], F32)
                nc.tensor.matmul(pse[0:100, :], wext[:, :], xt[:, :],
                                 start=True, stop=True)

                # --- h1 relu drains (bias fused); fp8 out for DoubleRow
                # mm2.  h1q layout [p, half, ksub, col]:
                # ksub 0 = h1 dims 0:128, ksub 1 = dims 128:256 ---
                h1q = h1p.tile([128, 2, 2, 512], FP8, tag="h1")
                nc.scalar.activation(h1q[:, :, 0, :], ps1t[:], AF.Relu,
                                     bias=b1[:, 0:1])
                nc.vector.tensor_scalar(h1q[:, :, 1, :], ps2t[:],
                                        b1[:, 1:2], 0.0, ALU.add, ALU.max)

                # --- extras: (z+bias)^2 ---
                stk = stkp.tile([128, 512], BF16)
                nc.scalar.activation(stk[0:100, :], pse[0:100, :], AF.Square,
                                     bias=bext[0:100, 0:1])

                # --- fm: accumulate +-0.5 coefficient reduction ---
                nc.tensor.matmul(pcol[:], wcoef[0:100, 16 * g:16 * g + 16],
                                 stk[0:100, :],
                                 start=(g == 0), stop=(g == NSUP - 1),
                                 skip_group_check=True)

                # --- mm2: h2pre*SW2 via fp8 DoubleRow, K=256 in one MM ---
                ph = pph.tile([128, 1024], F32)
                nc.tensor.matmul(ph[:, 0:512], w2q[:], h1q[:, 0, :, :],
                                 start=True, stop=True,
                                 perf_mode=mybir.MatmulPerfMode.DoubleRow)
                nc.tensor.matmul(ph[:, 512:1024], w2q[:], h1q[:, 1, :, :],
                                 start=True, stop=True,
                                 perf_mode=mybir.MatmulPerfMode.DoubleRow)

                # --- out drain split across ACT and DVE; host /SW2 ---
                outsb = outp.tile([128, 1024], BF16)
                nc.scalar.activation(outsb[:, 0:OSPLIT], ph[:, 0:OSPLIT],
                                     AF.Relu, bias=b2[:, 0:1])
                nc.vector.tensor_scalar(outsb[:, OSPLIT:1024],
                                        ph[:, OSPLIT:1024], b2[:, 0:1], 0.0,
                                        ALU.add, ALU.max)
                nc.sync.dma_start(out=hT_d[:, 1024 * g:1024 * (g + 1)],
                                  in_=outsb[:])

            # --- fm column drain, once ---
            colsb = colp.tile([16, 512], F32)
            nc.scalar.copy(colsb[:], pcol[:])
            nc.sync.dma_start(out=fmv_d[:], in_=colsb[:])

    nc.compile()
    return nc


def kernel(x, emb_tables, Wc, bc, Wf, bf, W1, b1, W2, b2):
    global _cached_nc, LAST_RESULT
    w, c0 = _precompute_weights(emb_tables, Wc, bc, Wf, bf, W1, b1, W2, b2)
    if _cached_nc is None:
        _cached_nc = _build_nc()
    nc = _cached_nc

    xtd = _pack_x(x)
    in_maps = []
    for i in range(NCORES):
        m = {"xtd": xtd[i]}
        m.update(w)
        in_maps.append(m)

    res = run_bass_kernel_spmd(nc, in_maps, list(range(NCORES)),
                               trace=TRACE, **TRACE_KW)
    LAST_RESULT = res
    out = np.empty((B, 129), np.float32)
    for i in range(NCORES):
        r = res.results[i]
        out[i * BS:(i + 1) * BS, 0] = (
            r["fmv"].astype(np.float32).reshape(-1) + c0)
        out[i * BS:(i + 1) * BS, 1:129] = (
            r["hT"].astype(np.float32).T * (1.0 / SW2))
    return out


# revision 11
# speedup vs baseline: 3.8472x; 1.0703x over previous
"""DeepFM Trainium2 kernel (8-core data-parallel), v2.

Math: x entries are binary {0,1}, so the per-feature embedding gather is
linear in x:  emb[b,f] = T0[f] + x[b,f]*(T1[f]-T0[f]).  The model folds to
    h1    = relu(x @ W1eff + b1eff)           # K=64 matmul
    h2    = relu(h1 @ W2.T + b2)
    fm    = x@lin0 + c0 + 0.5*||x@S+sbase||^2 - 0.5*||x@Wcont+bc||^2

Device structure per 1024-row superblock (A=rows 0:512, B=rows 512:1024;
x^T is prepared host-side as bf16 [128, 4096] with A-features on
partitions 0:64 and B-features on 64:128):
  - mm1: 4 matmuls K=64 (A/B pairs run concurrently via tile_position)
    -> two PSUM regions [128, 1024] (h1 dims 0:128, 128:256)
  - extras: ONE K=128 block-diag matmul -> pse[0:100] = per-half
    [sum_e(16) | cont(32) | lin+-(2)]; the linear fm term rides along as
    0.5*((z+.5)^2 - (z-.5)^2) = z through the square drain
  - drains: ACT relu-drains h1 chunk1 + squares extras; DVE relu-drains
    h1 chunk2 + the h2 output
  - fm: ONE K=100 matmul with +-0.5 coefficients accumulating into a
    persistent PSUM bank (cols 2g, 2g+1 select the superblock's rows)
  - mm2: 4 matmuls K=128 -> h2
Outputs: hT bf16 [128, 8192] (h2 transposed), fmv f32 [16, 512].
Host adds c0 to fm and transposes hT back. PSUM: 2+2+1+2+1 = 8 banks.
"""

import numpy as np
import ml_dtypes

import concourse.bass as bass
import concourse.tile as tile
from concourse import bacc, mybir
from concourse.bass_utils import run_bass_kernel_spmd

B = 65536
FEAT = 64
NUM_DISC = 62
D = 16
H1, H2 = 256, 128
NCORES = 8
BS = B // NCORES          # 8192 rows per core
NSUP = BS // 1024         # 8 superblocks of 1024 rows (512 "A" + 512 "B")

F32 = mybir.dt.float32
BF16 = mybir.dt.bfloat16
FP8 = mybir.dt.float8e4
AF = mybir.ActivationFunctionType
ALU = mybir.AluOpType
SW2 = 64.0                # fp8 scale on W2 (host divides hT by SW2)
OSPLIT = 320              # out-drain columns drained on ACT (rest on DVE)

TRACE = False
TRACE_KW = {}
LAST_RESULT = None

_cached_nc = None


def _precompute_weights(emb_tables, Wc, bc, Wf, bf, W1, b1, W2, b2):
    """Host-side weight folding, float64 for exactness."""
    T = np.asarray(emb_tables, np.float64)        # [62, 2, 16]
    Wc = np.asarray(Wc, np.float64)               # [32, 2]
    bc = np.asarray(bc, np.float64)               # [32]
    Wf = np.asarray(Wf, np.float64)               # [1, 64]
    bf = np.asarray(bf, np.float64)               # [1]
    W1 = np.asarray(W1, np.float64)               # [256, 1024]
    b1 = np.asarray(b1, np.float64)               # [256]
    W2 = np.asarray(W2, np.float64)               # [128, 256]
    b2 = np.asarray(b2, np.float64)               # [128]

    A = np.zeros((64, 1024))
    base = np.zeros(1024)
    for f in range(NUM_DISC):
        A[f, 16 * f:16 * f + 16] = T[f, 1] - T[f, 0]
        base[16 * f:16 * f + 16] = T[f, 0]
    A[62, 992:1024] = Wc[:, 0]
    A[63, 992:1024] = Wc[:, 1]
    base[992:1024] = bc

    W1eff = A @ W1.T                              # [64, 256]
    b1eff = base @ W1.T + b1                      # [256]
    S = A.reshape(64, 64, 16).sum(axis=1)         # [64, 16]
    sbase = base.reshape(64, 16).sum(axis=0)      # [16]
    Wcont = A[:, 992:1024]                        # [64, 32]

    q0 = (T[:, 0] ** 2).sum(axis=1)               # [62]
    q1 = (T[:, 1] ** 2).sum(axis=1)
    qlin = np.zeros(64)
    qlin[:NUM_DISC] = q1 - q0
    qconst = q0.sum()
    lin0 = Wf[0] - 0.5 * qlin                     # [64]
    c0 = bf[0] - 0.5 * qconst                     # scalar

    def dup(a):  # stack A-copy (parts 0:64) and B-copy (parts 64:128)
        return np.concatenate([a, a], axis=0)

    wmm1 = dup(W1eff).astype(ml_dtypes.bfloat16)           # [128, 256]

    # extras block: per half [S(16) | Wcont(32) | lin0 | lin0] = 50 cols
    blkA = np.concatenate(
        [S, Wcont, lin0[:, None], lin0[:, None]], axis=1)  # [64, 50]
    wext = np.zeros((128, 100))
    wext[0:64, 0:50] = blkA
    wext[64:128, 50:100] = blkA
    wext = wext.astype(ml_dtypes.bfloat16)

    # extras bias: sum_e -> sbase, cont -> bc, lin rows -> +-0.5
    bx = np.concatenate([sbase, bc, [0.5], [-0.5]])        # [50]
    bext = np.zeros((128, 1), np.float32)
    bext[0:50, 0] = bx
    bext[50:100, 0] = bx

    # fm coefficients: 0.5*sum_e^2 - 0.5*cont^2 + 0.5*((z+.5)^2-(z-.5)^2)
    cf = np.zeros(50)
    cf[0:16] = 0.5
    cf[16:48] = -0.5
    cf[48] = 0.5
    cf[49] = -0.5
    wcoef = np.zeros((128, 16 * NSUP))
    for g in range(NSUP):
        wcoef[0:50, 16 * g + 2 * g] = cf
        wcoef[50:100, 16 * g + 2 * g + 1] = cf
    wcoef = wcoef.astype(ml_dtypes.bfloat16)               # [128, 128]

    # mm2 stationary for fp8 DoubleRow: w2q[p, s, m] = SW2 * W2[m, 128s+p]
    w2q = SW2 * np.stack([W2[:, 0:128].T, W2[:, 128:256].T], axis=1)
    w2q = w2q.astype(ml_dtypes.float8_e4m3)                # [128, 2, 128]

    b1w = np.stack([b1eff[0:128], b1eff[128:256]], axis=1).astype(np.float32)
    b2w = (SW2 * b2)[:, None].astype(np.float32)           # [128, 1]

    w = dict(wmm1=wmm1, wext=wext, bextw=bext, wcoef=wcoef,
             w2q=w2q, b1w=b1w, b2w=b2w)
    return w, float(c0)


def _pack_x(x):
    """x [B, 64] f32 -> per-core xtd [128, 4096] bf16 with
    xtd[c][64*h + f, 512*g + j] = x[c*8192 + 1024*g + 512*h + j, f]."""
    xc = np.asarray(x, np.float32).reshape(NCORES, NSUP, 2, 512, FEAT)
    xt = xc.transpose(0, 2, 4, 1, 3).reshape(NCORES, 128, NSUP * 512)
    return np.ascontiguousarray(xt).astype(ml_dtypes.bfloat16)


def _build_nc():
    nc = bacc.Bacc(None, target_bir_lowering=False)

    xtd_d = nc.declare_dram_parameter("xtd", [128, 512 * NSUP], BF16,
                                      isOutput=False)
    wmm1_d = nc.declare_dram_parameter("wmm1", [128, 256], BF16, isOutput=False)
    wext_d = nc.declare_dram_parameter("wext", [128, 100], BF16, isOutput=False)
    wcoef_d = nc.declare_dram_parameter("wcoef", [128, 16 * NSUP], BF16,
                                        isOutput=False)
    w2q_d = nc.declare_dram_parameter("w2q", [128, 2, 128], FP8, isOutput=False)
    b1_d = nc.declare_dram_parameter("b1w", [128, 2], F32, isOutput=False)
    bext_d = nc.declare_dram_parameter("bextw", [128, 1], F32, isOutput=False)
    b2_d = nc.declare_dram_parameter("b2w", [128, 1], F32, isOutput=False)
    hT_d = nc.declare_dram_parameter("hT", [128, BS], BF16, isOutput=True)
    fmv_d = nc.declare_dram_parameter("fmv", [16, 512], F32, isOutput=True)

    with tile.TileContext(nc) as tc:
        from contextlib import ExitStack
        with ExitStack() as ctx:
            constp = ctx.enter_context(tc.tile_pool(name="const", bufs=1))
            xtp = ctx.enter_context(tc.tile_pool(name="xt", bufs=NSUP))
            h1p = ctx.enter_context(tc.tile_pool(name="h1", bufs=4))
            stkp = ctx.enter_context(tc.tile_pool(name="stk", bufs=2))
            outp = ctx.enter_context(tc.tile_pool(name="outsb", bufs=4))
            colp = ctx.enter_context(tc.tile_pool(name="colsb", bufs=1))
            pp1 = ctx.enter_context(
                tc.tile_pool(name="ps1", bufs=1, space=bass.MemorySpace.PSUM))
            pp2 = ctx.enter_context(
                tc.tile_pool(name="ps2", bufs=1, space=bass.MemorySpace.PSUM))
            ppe = ctx.enter_context(
                tc.tile_pool(name="pse", bufs=1, space=bass.MemorySpace.PSUM))
            pph = ctx.enter_context(
                tc.tile_pool(name="psh", bufs=1, space=bass.MemorySpace.PSUM))
            ppc = ctx.enter_context(
                tc.tile_pool(name="psc", bufs=1, space=bass.MemorySpace.PSUM))

            wmm1 = constp.tile([128, 256], BF16)
            nc.sync.dma_start(out=wmm1[:], in_=wmm1_d[:])
            wext = constp.tile([128, 100], BF16)
            nc.sync.dma_start(out=wext[:], in_=wext_d[:])
            wcoef = constp.tile([128, 16 * NSUP], BF16)
            nc.sync.dma_start(out=wcoef[:], in_=wcoef_d[:])
            w2q = constp.tile([128, 2, 128], FP8)
            nc.sync.dma_start(out=w2q[:], in_=w2q_d[:])
            b1 = constp.tile([128, 2], F32)
            nc.sync.dma_start(out=b1[:], in_=b1_d[:])
            bext = constp.tile([128, 1], F32)
            nc.sync.dma_start(out=bext[:], in_=bext_d[:])
            b2 = constp.tile([128, 1], F32)
            nc.sync.dma_start(out=b2[:], in_=b2_d[:])

            # fm accumulator, persistent across all superblocks
            pcol = ppc.tile([16, 512], F32)

            # hoist all input loads so no store can queue ahead of them
            xts = []
            for g in range(NSUP):
                xt = xtp.tile([128, 512], BF16)
                nc.sync.dma_start(out=xt[:],
                                  in_=xtd_d[:, 512 * g:512 * (g + 1)])
                xts.append(xt)

            # Software-pipelined by one superblock: iteration g emits
            # mm1/extras/h1-drains/square for g, but mm2 + out-drain +
            # store for g-1.  This keeps each engine's strict-FIFO queue
            # free of cross-engine round-trip waits (e.g. DVE's
            # h1c1(g+1) no longer queues behind out(g), which would wait
            # on mm2(g) on the PE).
            h1qs = [None, None]
            def _mm2_and_out(gp):
                h1q = h1qs[gp % 2]
                ph = pph.tile([128, 1024], F32)
                nc.tensor.matmul(ph[:, 0:512], w2q[:], h1q[:, 0, :, :],
                                 start=True, stop=True,
                                 perf_mode=mybir.MatmulPerfMode.DoubleRow)
                nc.tensor.matmul(ph[:, 512:1024], w2q[:], h1q[:, 1, :, :],
                                 start=True, stop=True,
                                 perf_mode=mybir.MatmulPerfMode.DoubleRow)
                outsb = outp.tile([128, 1024], BF16)
                nc.scalar.activation(outsb[:, 0:OSPLIT], ph[:, 0:OSPLIT],
                                     AF.Relu, bias=b2[:, 0:1])
                nc.vector.tensor_scalar(outsb[:, OSPLIT:1024],
                                        ph[:, OSPLIT:1024], b2[:, 0:1], 0.0,
                                        ALU.add, ALU.max)
                nc.sync.dma_start(out=hT_d[:, 1024 * gp:1024 * (gp + 1)],
                                  in_=outsb[:])

            for g in range(NSUP):
                xt = xts[g]

                # --- mm1: h1pre, A/B halves concurrent per chunk ---
                ps1t = pp1.tile([128, 2, 512], F32)
                nc.tensor.matmul(ps1t[:, 0, :], wmm1[0:64, 0:128],
                                 xt[0:64, :], start=True, stop=True)
                nc.tensor.matmul(ps1t[:, 1, :], wmm1[64:128, 0:128],
                                 xt[64:128, :], start=True, stop=True,
                                 tile_position=(64, 0))
                ps2t = pp2.tile([128, 2, 512], F32)
                nc.tensor.matmul(ps2t[:, 0, :], wmm1[0:64, 128:256],
                                 xt[0:64, :], start=True, stop=True)
                nc.tensor.matmul(ps2t[:, 1, :], wmm1[64:128, 128:256],
                                 xt[64:128, :], start=True, stop=True,
                                 tile_position=(64, 0))

                # --- extras: one K=128 block-diag matmul ---
                pse = ppe.tile([128, 512], F32)
                nc.tensor.matmul(pse[0:100, :], wext[:, :], xt[:, :],
                                 start=True, stop=True)

                # --- h1 relu drains (bias fused); fp8 out for DoubleRow
                # mm2.  h1q layout [p, half, ksub, col]:
                # ksub 0 = h1 dims 0:128, ksub 1 = dims 128:256 ---
                h1q = h1p.tile([128, 2, 2, 512], FP8, tag="h1")
                h1qs[g % 2] = h1q
                nc.scalar.activation(h1q[:, :, 0, :], ps1t[:], AF.Relu,
                                     bias=b1[:, 0:1])
                nc.vector.tensor_scalar(h1q[:, :, 1, :], ps2t[:],
                                        b1[:, 1:2], 0.0, ALU.add, ALU.max)

                # --- previous superblock's mm2 + out drain + store ---
                if g > 0:
                    _mm2_and_out(g - 1)

                # --- extras: (z+bias)^2 ---
                stk = stkp.tile([128, 512], BF16)
                nc.scalar.activation(stk[0:100, :], pse[0:100, :], AF.Square,
                                     bias=bext[0:100, 0:1])

                # --- fm: accumulate +-0.5 coefficient reduction ---
                nc.tensor.matmul(pcol[:], wcoef[0:100, 16 * g:16 * g + 16],
                                 stk[0:100, :],
                                 start=(g == 0), stop=(g == NSUP - 1),
                                 skip_group_check=True)

            _mm2_and_out(NSUP - 1)

            # --- fm column drain, once ---
            colsb = colp.tile([16, 512], F32)
            nc.scalar.copy(colsb[:], pcol[:])
            nc.sync.dma_start(out=fmv_d[:], in_=colsb[:])

    nc.compile()
    return nc


def kernel(x, emb_tables, Wc, bc, Wf, bf, W1, b1, W2, b2):
    global _cached_nc, LAST_RESULT
    w, c0 = _precompute_weights(emb_tables, Wc, bc, Wf, bf, W1, b1, W2, b2)
    if _cached_nc is None:
        _cached_nc = _build_nc()
    nc = _cached_nc

    xtd = _pack_x(x)
    in_maps = []
    for i in range(NCORES):
        m = {"xtd": xtd[i]}
        m.update(w)
        in_maps.append(m)

    res = run_bass_kernel_spmd(nc, in_maps, list(range(NCORES)),
                               trace=TRACE, **TRACE_KW)
    LAST_RESULT = res
    out = np.empty((B, 129), np.float32)
    for i in range(NCORES):
        r = res.results[i]
        out[i * BS:(i + 1) * BS, 0] = (
            r["fmv"].astype(np.float32).reshape(-1) + c0)
        out[i * BS:(i + 1) * BS, 1:129] = (
            r["hT"].astype(np.float32).T * (1.0 / SW2))
    return out


# revision 17
# speedup vs baseline: 3.8830x; 1.0093x over previous
"""DeepFM Trainium2 kernel (8-core data-parallel), v2.

Math: x entries are binary {0,1}, so the per-feature embedding gather is
linear in x:  emb[b,f] = T0[f] + x[b,f]*(T1[f]-T0[f]).  The model folds to
    h1    = relu(x @ W1eff + b1eff)           # K=64 matmul
    h2    = relu(h1 @ W2.T + b2)
    fm    = x@lin0 + c0 + 0.5*||x@S+sbase||^2 - 0.5*||x@Wcont+bc||^2

Device structure per 1024-row superblock (A=rows 0:512, B=rows 512:1024;
x^T is prepared host-side as bf16 [128, 4096] with A-features on
partitions 0:64 and B-features on 64:128):
  - mm1: 4 matmuls K=64 (A/B pairs run concurrently via tile_position)
    -> two PSUM regions [128, 1024] (h1 dims 0:128, 128:256)
  - extras: ONE K=128 block-diag matmul -> pse[0:100] = per-half
    [sum_e(16) | cont(32) | lin+-(2)]; the linear fm term rides along as
    0.5*((z+.5)^2 - (z-.5)^2) = z through the square drain
  - drains: ACT relu-drains h1 chunk1 + squares extras; DVE relu-drains
    h1 chunk2 + the h2 output
  - fm: ONE K=100 matmul with +-0.5 coefficients accumulating into a
    persistent PSUM bank (cols 2g, 2g+1 select the superblock's rows)
  - mm2: 4 matmuls K=128 -> h2
Outputs: hT bf16 [128, 8192] (h2 transposed), fmv f32 [16, 512].
Host adds c0 to fm and transposes hT back. PSUM: 2+2+1+2+1 = 8 banks.
"""

import numpy as np
import ml_dtypes

import concourse.bass as bass
import concourse.tile as tile
from concourse import bacc, mybir
from concourse.bass_utils import run_bass_kernel_spmd

B = 65536
FEAT = 64
NUM_DISC = 62
D = 16
H1, H2 = 256, 128
NCORES = 8
BS = B // NCORES          # 8192 rows per core
NSUP = BS // 1024         # 8 superblocks of 1024 rows (512 "A" + 512 "B")

F32 = mybir.dt.float32
BF16 = mybir.dt.bfloat16
FP8 = mybir.dt.float8e4
AF = mybir.ActivationFunctionType
ALU = mybir.AluOpType
SW2 = 64.0                # fp8 scale on W2 (host divides hT by SW2)
OSPLIT = 288              # out-drain columns drained on ACT (rest on DVE)

TRACE = False
TRACE_KW = {}
LAST_RESULT = None

_cached_nc = None


def _precompute_weights(emb_tables, Wc, bc, Wf, bf, W1, b1, W2, b2):
    """Host-side weight folding, float64 for exactness."""
    T = np.asarray(emb_tables, np.float64)        # [62, 2, 16]
    Wc = np.asarray(Wc, np.float64)               # [32, 2]
    bc = np.asarray(bc, np.float64)               # [32]
    Wf = np.asarray(Wf, np.float64)               # [1, 64]
    bf = np.asarray(bf, np.float64)               # [1]
    W1 = np.asarray(W1, np.float64)               # [256, 1024]
    b1 = np.asarray(b1, np.float64)               # [256]
    W2 = np.asarray(W2, np.float64)               # [128, 256]
    b2 = np.asarray(b2, np.float64)               # [128]

    A = np.zeros((64, 1024))
    base = np.zeros(1024)
    for f in range(NUM_DISC):
        A[f, 16 * f:16 * f + 16] = T[f, 1] - T[f, 0]
        base[16 * f:16 * f + 16] = T[f, 0]
    A[62, 992:1024] = Wc[:, 0]
    A[63, 992:1024] = Wc[:, 1]
    base[992:1024] = bc

    W1eff = A @ W1.T                              # [64, 256]
    b1eff = base @ W1.T + b1                      # [256]
    S = A.reshape(64, 64, 16).sum(axis=1)         # [64, 16]
    sbase = base.reshape(64, 16).sum(axis=0)      # [16]
    Wcont = A[:, 992:1024]                        # [64, 32]

    q0 = (T[:, 0] ** 2).sum(axis=1)               # [62]
    q1 = (T[:, 1] ** 2).sum(axis=1)
    qlin = np.zeros(64)
    qlin[:NUM_DISC] = q1 - q0
    qconst = q0.sum()
    lin0 = Wf[0] - 0.5 * qlin                     # [64]
    c0 = bf[0] - 0.5 * qconst                     # scalar

    def dup(a):  # stack A-copy (parts 0:64) and B-copy (parts 64:128)
        return np.concatenate([a, a], axis=0)

    wmm1 = dup(W1eff).astype(ml_dtypes.bfloat16)           # [128, 256]

    # extras block: per half [S(16) | Wcont(32) | lin0 | lin0] = 50 cols
    blkA = np.concatenate(
        [S, Wcont, lin0[:, None], lin0[:, None]], axis=1)  # [64, 50]
    wext = np.zeros((128, 100))
    wext[0:64, 0:50] = blkA
    wext[64:128, 50:100] = blkA
    wext = wext.astype(ml_dtypes.bfloat16)

    # extras bias: sum_e -> sbase, cont -> bc, lin rows -> +-0.5
    bx = np.concatenate([sbase, bc, [0.5], [-0.5]])        # [50]
    bext = np.zeros((128, 1), np.float32)
    bext[0:50, 0] = bx
    bext[50:100, 0] = bx

    # fm coefficients: 0.5*sum_e^2 - 0.5*cont^2 + 0.5*((z+.5)^2-(z-.5)^2)
    cf = np.zeros(50)
    cf[0:16] = 0.5
    cf[16:48] = -0.5
    cf[48] = 0.5
    cf[49] = -0.5
    wcoef = np.zeros((128, 16 * NSUP))
    for g in range(NSUP):
        wcoef[0:50, 16 * g + 2 * g] = cf
        wcoef[50:100, 16 * g + 2 * g + 1] = cf
    wcoef = wcoef.astype(ml_dtypes.bfloat16)               # [128, 128]

    # mm2 stationary for fp8 DoubleRow: w2q[p, s, m] = SW2 * W2[m, 128s+p]
    w2q = SW2 * np.stack([W2[:, 0:128].T, W2[:, 128:256].T], axis=1)
    w2q = w2q.astype(ml_dtypes.float8_e4m3)                # [128, 2, 128]

    b1w = np.stack([b1eff[0:128], b1eff[128:256]], axis=1)

    # single bf16 weight pack: [wmm1(256) | wext(100) | wcoef(128)]
    wbf = np.concatenate(
        [wmm1.astype(np.float64), wext.astype(np.float64),
         wcoef.astype(np.float64)], axis=1)                # [128, 484]
    # single f32 bias pack: [b1(2) | bext(1) | b2*SW2(1)]
    bias = np.zeros((128, 4))
    bias[:, 0:2] = b1w
    bias[:, 2:3] = bext
    bias[:, 3] = SW2 * b2
    w = dict(wbf=wbf.astype(ml_dtypes.bfloat16), w2q=w2q,
             bias=bias.astype(np.float32))
    return w, float(c0)


def _pack_x(x):
    """x [B, 64] f32 -> per-core xtd [128, 4096] bf16 with
    xtd[c][64*h + f, 512*g + j] = x[c*8192 + 1024*g + 512*h + j, f]."""
    xc = np.asarray(x, np.float32).reshape(NCORES, NSUP, 2, 512, FEAT)
    xt = xc.transpose(0, 2, 4, 1, 3).reshape(NCORES, 128, NSUP * 512)
    return np.ascontiguousarray(xt).astype(ml_dtypes.bfloat16)


def _build_nc():
    nc = bacc.Bacc(None, target_bir_lowering=False)

    xtd_d = nc.declare_dram_parameter("xtd", [128, 512 * NSUP], BF16,
                                      isOutput=False)
    wbf_d = nc.declare_dram_parameter("wbf", [128, 484], BF16, isOutput=False)
    w2q_d = nc.declare_dram_parameter("w2q", [128, 2, 128], FP8, isOutput=False)
    bias_d = nc.declare_dram_parameter("bias", [128, 4], F32, isOutput=False)
    hT_d = nc.declare_dram_parameter("hT", [128, BS], BF16, isOutput=True)
    fmv_d = nc.declare_dram_parameter("fmv", [16, 512], F32, isOutput=True)

    with tile.TileContext(nc) as tc:
        from contextlib import ExitStack
        with ExitStack() as ctx:
            constp = ctx.enter_context(tc.tile_pool(name="const", bufs=1))
            xtp = ctx.enter_context(tc.tile_pool(name="xt", bufs=1))
            h1p = ctx.enter_context(tc.tile_pool(name="h1", bufs=4))
            stkp = ctx.enter_context(tc.tile_pool(name="stk", bufs=2))
            outp = ctx.enter_context(tc.tile_pool(name="outsb", bufs=4))
            colp = ctx.enter_context(tc.tile_pool(name="colsb", bufs=1))
            pp1 = ctx.enter_context(
                tc.tile_pool(name="ps1", bufs=1, space=bass.MemorySpace.PSUM))
            pp2 = ctx.enter_context(
                tc.tile_pool(name="ps2", bufs=1, space=bass.MemorySpace.PSUM))
            ppe = ctx.enter_context(
                tc.tile_pool(name="pse", bufs=1, space=bass.MemorySpace.PSUM))
            pph = ctx.enter_context(
                tc.tile_pool(name="psh", bufs=1, space=bass.MemorySpace.PSUM))
            ppc = ctx.enter_context(
                tc.tile_pool(name="psc", bufs=1, space=bass.MemorySpace.PSUM))

            # weights/biases: 3 DMAs on the scalar queue (idle pre-compute)
            wbf = constp.tile([128, 484], BF16)
            nc.scalar.dma_start(out=wbf[:], in_=wbf_d[:])
            w2q = constp.tile([128, 2, 128], FP8)
            nc.scalar.dma_start(out=w2q[:], in_=w2q_d[:])
            biast = constp.tile([128, 4], F32)
            nc.scalar.dma_start(out=biast[:], in_=bias_d[:])
            wmm1 = wbf[:, 0:256]
            wext = wbf[:, 256:356]
            wcoef = wbf[:, 356:484]
            b1 = biast[:, 0:2]
            bext = biast[:, 2:3]
            b2 = biast[:, 3:4]

            # fm accumulator, persistent across all superblocks
            pcol = ppc.tile([16, 512], F32)

            # PE warm-up: dummy matmuls on a zeroed tile while the input
            # DMAs stream in, so HAM reaches 8/8 before real work starts.
            # They write the pcol bank, which g=0's start=True then clears.
            dummy = constp.tile([64, 512], BF16)
            nc.gpsimd.memset(dummy[:], 0.0)
            for _ in range(8):
                nc.tensor.matmul(pcol[:], dummy[:, 0:16], dummy[:, :],
                                 start=True, stop=True,
                                 skip_group_check=True)

            # input x: one tile, two DMAs (first superblock alone so
            # compute starts early, remaining seven in one transfer)
            xtall = xtp.tile([128, NSUP, 512], BF16)
            nc.sync.dma_start(out=xtall[:, 0, :], in_=xtd_d[:, 0:512])
            nc.sync.dma_start(out=xtall[:, 1:NSUP, :],
                              in_=xtd_d[:, 512:512 * NSUP])
            xts = [xtall[:, g, :] for g in range(NSUP)]

            # Software-pipelined by one superblock: iteration g emits
            # mm1/extras/h1-drains/square for g, but mm2 + out-drain +
            # store for g-1.  This keeps each engine's strict-FIFO queue
            # free of cross-engine round-trip waits (e.g. DVE's
            # h1c1(g+1) no longer queues behind out(g), which would wait
            # on mm2(g) on the PE).
            h1qs = [None, None]
            def _mm2_and_out(gp):
                h1q = h1qs[gp % 2]
                ph = pph.tile([128, 1024], F32)
                nc.tensor.matmul(ph[:, 0:512], w2q[:], h1q[:, 0, :, :],
                                 start=True, stop=True,
                                 perf_mode=mybir.MatmulPerfMode.DoubleRow)
                nc.tensor.matmul(ph[:, 512:1024], w2q[:], h1q[:, 1, :, :],
                                 start=True, stop=True,
                                 perf_mode=mybir.MatmulPerfMode.DoubleRow)
                outsb = outp.tile([128, 1024], BF16)
                nc.scalar.activation(outsb[:, 0:OSPLIT], ph[:, 0:OSPLIT],
                                     AF.Relu, bias=b2[:, 0:1])
                nc.vector.tensor_scalar(outsb[:, OSPLIT:1024],
                                        ph[:, OSPLIT:1024], b2[:, 0:1], 0.0,
                                        ALU.add, ALU.max)
                nc.sync.dma_start(out=hT_d[:, 1024 * gp:1024 * (gp + 1)],
                                  in_=outsb[:])

            for g in range(NSUP):
                xt = xts[g]

                # --- mm1: h1pre, A/B halves concurrent per chunk ---
                ps1t = pp1.tile([128, 2, 512], F32)
                nc.tensor.matmul(ps1t[:, 0, :], wmm1[0:64, 0:128],
                                 xt[0:64, :], start=True, stop=True)
                nc.tensor.matmul(ps1t[:, 1, :], wmm1[64:128, 0:128],
                                 xt[64:128, :], start=True, stop=True,
                                 tile_position=(64, 0))
                ps2t = pp2.tile([128, 2, 512], F32)
                nc.tensor.matmul(ps2t[:, 0, :], wmm1[0:64, 128:256],
                                 xt[0:64, :], start=True, stop=True)
                nc.tensor.matmul(ps2t[:, 1, :], wmm1[64:128, 128:256],
                                 xt[64:128, :], start=True, stop=True,
                                 tile_position=(64, 0))

                # --- extras: one K=128 block-diag matmul ---
                pse = ppe.tile([128, 512], F32)
                nc.tensor.matmul(pse[0:100, :], wext[:, :], xt[:, :],
                                 start=True, stop=True)

                # --- h1 relu drains (bias fused); fp8 out for DoubleRow
                # mm2.  h1q layout [p, half, ksub, col]:
                # ksub 0 = h1 dims 0:128, ksub 1 = dims 128:256 ---
                h1q = h1p.tile([128, 2, 2, 512], FP8, tag="h1")
                h1qs[g % 2] = h1q
                nc.scalar.activation(h1q[:, :, 0, :], ps1t[:], AF.Relu,
                                     bias=b1[:, 0:1])
                nc.vector.tensor_scalar(h1q[:, :, 1, :], ps2t[:],
                                        b1[:, 1:2], 0.0, ALU.add, ALU.max)

                # --- previous superblock's mm2 + out drain + store ---
                if g > 0:
                    _mm2_and_out(g - 1)

                # --- extras: (z+bias)^2 ---
                stk = stkp.tile([128, 512], BF16)
                nc.scalar.activation(stk[0:100, :], pse[0:100, :], AF.Square,
                                     bias=bext[0:100, 0:1])

                # --- fm: accumulate +-0.5 coefficient reduction ---
                nc.tensor.matmul(pcol[:], wcoef[0:100, 16 * g:16 * g + 16],
                                 stk[0:100, :],
                                 start=(g == 0), stop=(g == NSUP - 1),
                                 skip_group_check=True)

            _mm2_and_out(NSUP - 1)

            # --- fm column drain, once ---
            colsb = colp.tile([16, 512], F32)
            nc.scalar.copy(colsb[:], pcol[:])
            nc.sync.dma_start(out=fmv_d[:], in_=colsb[:])

    nc.compile()
    return nc


def kernel(x, emb_tables, Wc, bc, Wf, bf, W1, b1, W2, b2):
    global _cached_nc, LAST_RESULT
    w, c0 = _precompute_weights(emb_tables, Wc, bc, Wf, bf, W1, b1, W2, b2)
    if _cached_nc is None:
        _cached_nc = _build_nc()
    nc = _cached_nc

    xtd = _pack_x(x)
    in_maps = []
    for i in range(NCORES):
        m = {"xtd": xtd[i]}
        m.update(w)
        in_maps.append(m)

    res = run_bass_kernel_spmd(nc, in_maps, list(range(NCORES)),
                               trace=TRACE, **TRACE_KW)
    LAST_RESULT = res
    out = np.empty((B, 129), np.float32)
    for i in range(NCORES):
        r = res.results[i]
        out[i * BS:(i + 1) * BS, 0] = (
            r["fmv"].astype(np.float32).reshape(-1) + c0)
        out[i * BS:(i + 1) * BS, 1:129] = (
            r["hT"].astype(np.float32).T * (1.0 / SW2))
    return out
